# revision 1
# baseline (speedup 1.0000x reference)
"""Masked-linear kernel for Trainium2 (8 NeuronCores).

Computes out = data @ (weight * w_mask)^T + bias_p with
  data   [4, 2048, 4096] fp32
  weight [4096, 4096]    fp32
  w_mask [4096, 4096]    fp32
  bias_p [4096]          fp32
  out    [4, 2048, 4096] fp32

Sharding: 2D grid over 8 cores — 4 shards of out-features (N_C=1024) x
2 shards of tokens (M_C=4096). Weight/mask/bias are sliced per n-shard,
data per m-shard; each core computes its [M_C, N_C] output block.

Per core: the masked weight is built on-chip (DVE multiply), transposed
into k-major layout via PE-transpose, and kept resident in SBUF
([128, 32, 1024] = 16.8 MB). Data tiles are PE-transposed per m-tile and
fed as the stationary matmul operand. Matmuls run in float32r mode
(fp32 operands truncated to ~FP22, fp32 accumulate) which streams at
1 cycle/row like bf16; set BASS_KERNEL_DTYPE=f32 for a ~4x-slower
near-exact fp32 fallback. Bias is added during PSUM eviction.

Schedule: the weight build (DMA-bound, ~93us) is software-pipelined with
the first two m-tiles' matmuls, which fire per 512-wide k-chunk as that
chunk of wmT completes; the remaining m-tiles run as a flat pipeline
over (m-tile, k-chunk) groups in which matmuls lag transposes by 3
groups, giving a gapless PE stream of 4 transposes + 8 matmuls per
group with the psum->sbuf evictions alternating between DVE and ACT.
Cost-model (TimelineSim) estimate: ~624 us/core; PE busy ~547 us of
which 437 us is the matmul roofline.
"""

import os
import sys

if "/opt/trn_rl_repo" not in sys.path:
    sys.path.insert(0, "/opt/trn_rl_repo")

import numpy as np

import concourse.bass as bass  # noqa: F401  (import registers bass types)
import concourse.mybir as mybir
import concourse.tile as tile
from concourse import bacc
from concourse.bass_utils import run_bass_kernel_spmd
from concourse.masks import make_identity

# Problem shape (hardcoded per harness contract)
M_TOT = 8192          # 4 * 2048 tokens
K = 4096              # d_in
N_TOT = 4096          # d_out

N_CORES = 8
N_SHARDS = 4          # shards of out-features
M_SHARDS = 2          # shards of tokens
N_C = N_TOT // N_SHARDS   # 1024 out-features per core
M_C = M_TOT // M_SHARDS   # 4096 tokens per core

P = 128
KO = K // P           # 32 k-blocks of 128
MT = M_C // P         # 32 m-tiles of 128 tokens
NH = N_C // 512       # 2 psum-width groups

F32 = mybir.dt.float32

# "f32r" = float32r (FP22-truncated operands, full-rate PE stream);
# "f32"  = true fp32 (4 PE passes, ~4x slower, ~fp32-exact).
DTYPE_MODE = os.environ.get("BASS_KERNEL_DTYPE", "f32r")

LAST_RESULT = None    # BassKernelResults of the most recent run (for test.py)


def _mm_dt():
    return mybir.dt.float32r if DTYPE_MODE == "f32r" else mybir.dt.float32


def _build_program(repeat=1):
    nc = bacc.Bacc("TRN2", target_bir_lowering=False, debug=False,
                   num_devices=N_CORES)

    mmdt0 = _mm_dt()
    # Inputs feeding the PE are declared float32r end-to-end (bit-identical
    # to fp32 in DRAM); this lets the PE-transposes run in f32r mode
    # (1.5 cyc/row vs 2.0 for fp32) and satisfies the verifier's
    # rounded-producer rule. Numerics are unchanged: the main matmul
    # truncates operands to FP22 either way.
    data_d = nc.dram_tensor("data", [M_C, K], mmdt0, kind="ExternalInput").ap()
    w_d = nc.dram_tensor("w", [N_C, K], mmdt0, kind="ExternalInput").ap()
    mask_d = nc.dram_tensor("mask", [N_C, K], mmdt0, kind="ExternalInput").ap()
    bias_d = nc.dram_tensor("bias", [P, N_C], F32, kind="ExternalInput").ap()
    out_d = nc.dram_tensor("out", [M_C, N_C], F32, kind="ExternalOutput").ap()

    mmdt = _mm_dt()
    HK = K // 2            # 2048, half of k per dnat tile

    with tile.TileContext(nc) as tc:
        with (
            tc.tile_pool(name="const", bufs=1) as const_pool,
            tc.tile_pool(name="wm_res", bufs=1) as wm_res,
            tc.tile_pool(name="wload", bufs=3) as wload,
            tc.tile_pool(name="dnat", bufs=int(os.environ.get("KP_DNAT", "4"))) as dpool,
            tc.tile_pool(name="dT", bufs=int(os.environ.get("KP_DT", "5"))) as dTpool,
            tc.tile_pool(name="outp", bufs=2) as opool,
            tc.tile_pool(name="pst", bufs=int(os.environ.get("KP_PST", "3")), space="PSUM") as pst,
            tc.tile_pool(name="psmm0", bufs=3, space="PSUM") as psmm0,
            tc.tile_pool(name="psmm1", bufs=2, space="PSUM") as psmm1,
        ):
            for _rep in range(repeat):
                idn_f = const_pool.tile([P, P], F32, name="idn_f")
                make_identity(nc, idn_f)
                # f32r view of the identity via a DVE copy (a legal
                # "rounded" producer for the f32r transpose matmuls)
                idn = const_pool.tile([P, P], mmdt, name="idn")
                nc.vector.tensor_copy(idn[:], idn_f[:])

                bias_sb = const_pool.tile([P, N_C], F32, name="bias_sb")
                nc.sync.dma_start(bias_sb[:], bias_d)

                # Resident masked-weight, k-major: wmT[p=k_in, k_o, n]
                # (float32r dtype: the evicting copy rounds to FP22 on write,
                # which the BIR verifier requires for FP32r matmul operands)
                wmT = wm_res.tile([P, KO, N_C], mmdt, name="wmT")

                def load_half(mt, h):
                    """DMA half h (k cols h*2048..) of m-tile mt."""
                    dnat = dpool.tile([P, HK], mmdt, name="dnat", tag="dnat")
                    nc.sync.dma_start(
                        dnat[:], data_d[mt * P:(mt + 1) * P, h * HK:(h + 1) * HK])
                    return dnat

                def alloc_pmm(nhs=(0, 1)):
                    pools = {0: psmm0, 1: psmm1}
                    return {
                        nh: pools[nh].tile([P, 512], F32, name="pmm",
                                           tag=f"pmm{nh}")
                        for nh in nhs
                    }

                def emit_transposes(dnat_half, kc, use_act=False):
                    """PE-transpose one kc chunk (4 k-blocks) of one m-tile;
                    returns the SBUF tile with the k-major data. The psum->sbuf
                    eviction alternates between DVE and ACT so neither engine's
                    drain rate limits the PE."""
                    ps = pst.tile([P, 512], F32, name="psB", tag="pst")
                    base = (kc % 4) * 512
                    for j in range(4):
                        nc.tensor.transpose(
                            ps[:, j * P:(j + 1) * P].bitcast(mmdt),
                            dnat_half[:, base + j * P:base + (j + 1) * P],
                            idn[:],
                        )
                    dTt = dTpool.tile([P, 4, P], mmdt, name="dTt", tag="dTt")
                    ps_v = ps[:].rearrange("p (j n) -> p j n", j=4)
                    if use_act:
                        nc.scalar.copy(dTt[:], ps_v)
                    else:
                        nc.vector.tensor_copy(dTt[:], ps_v)
                    return dTt

                def emit_mms(dTt, kc, pmm, nhs=(0, 1)):
                    for j in range(4):
                        ko = kc * 4 + j
                        for nh in nhs:
                            nc.tensor.matmul(
                                pmm[nh][:],
                                dTt[:, j, :],
                                wmT[:, ko, nh * 512:(nh + 1) * 512],
                                start=(ko == 0),
                                stop=(ko == KO - 1),
                            )

                def emit_kc_group(dnat_half, kc, pmm):
                    emit_mms(emit_transposes(dnat_half, kc), kc, pmm)

                def emit_evict(mt, pmm, nhs=(0, 1)):
                    for nh in nhs:
                        ot = opool.tile([P, 512], F32, name="ot", tag="ot")
                        nc.vector.tensor_add(
                            ot[:], pmm[nh][:], bias_sb[:, nh * 512:(nh + 1) * 512])
                        nc.sync.dma_start(
                            out_d[mt * P:(mt + 1) * P, nh * 512:(nh + 1) * 512],
                            ot[:])

                # ---- Phase A, software-pipelined with the first PIPE m-tiles ----
                # Flat stream over the 64 weight blocks: each block's compute
                # (mult, 4 transposes, wmT evict) lags its DMA by AL blocks so
                # the chain runs at DMA rate, not at serial-latency rate. The
                # first PIPE m-tiles' matmuls run per kc chunk as soon as that
                # chunk of wmT is complete.
                from collections import deque
                PIPE = int(os.environ.get("KP_PIPE", "2"))
                AL = 2
                ablocks = [(kc, no) for kc in range(K // 512)
                           for no in range(N_C // P)]

                def load_wm(kc, no):
                    wt = wload.tile([P, 512], mmdt, name="wt", tag="wt")
                    mt_ = wload.tile([P, 512], mmdt, name="mt", tag="mt")
                    nc.sync.dma_start(
                        wt[:], w_d[no * P:(no + 1) * P, kc * 512:(kc + 1) * 512])
                    nc.sync.dma_start(
                        mt_[:], mask_d[no * P:(no + 1) * P, kc * 512:(kc + 1) * 512])
                    return wt, mt_

                def compute_block(kc, no, wt, mt_):
                    nc.vector.tensor_mul(wt[:], wt[:], mt_[:])
                    ps = pst.tile([P, 512], F32, name="psa", tag="pst")
                    for j in range(4):
                        nc.tensor.transpose(
                            ps[:, j * P:(j + 1) * P].bitcast(mmdt),
                            wt[:, j * P:(j + 1) * P],
                            idn[:],
                        )
                    # psum [k=128, (j, n=128)] -> wmT[:, kc*4+j, no*128:+128]
                    nc.scalar.copy(
                        wmT[:, kc * 4:(kc + 1) * 4, no * P:(no + 1) * P],
                        ps[:].rearrange("p (j n) -> p j n", j=4),
                    )

                # Optionally one extra "half" early tile that only computes
                # its nh=0 psum during phase A (third pmm0 bank); its nh=1
                # half runs at the head of phase B. Off by default: it
                # modeled slower (extra phase-A DMA contention).
                HALF = os.environ.get("KP_HALF", "0") == "1"
                HP = PIPE + (1 if HALF else 0)
                early_pmm = [alloc_pmm((0, 1)) for _ in range(PIPE)]
                if HALF:
                    early_pmm.append(alloc_pmm((0,)))
                early_h = [[None, None] for _ in range(HP)]
                pend = deque()

                early_dT = {}

                def phase_a_step(i):
                    kc2, no2 = ablocks[i]
                    compute_block(kc2, no2, *pend.popleft())
                    # wmT[:, kc2 chunk, nh half] complete after 4 no-blocks;
                    # fire the early m-tiles' matmuls per half-chunk.
                    if no2 == 3:
                        for emt in range(HP):
                            dTt = emit_transposes(
                                early_h[emt][kc2 // 4], kc2, use_act=emt % 2)
                            if emt < PIPE:
                                early_dT[emt] = dTt
                            emit_mms(dTt, kc2, early_pmm[emt], nhs=(0,))
                    elif no2 == N_C // P - 1:
                        for emt in range(PIPE):
                            emit_mms(early_dT.pop(emt), kc2,
                                     early_pmm[emt], nhs=(1,))

                for i, (kc, no) in enumerate(ablocks):
                    pend.append(load_wm(kc, no))
                    if i == 1:
                        for emt in range(HP):
                            early_h[emt][0] = load_half(emt, 0)
                    if i >= 26 and i % 2 == 0 and (i - 26) // 2 < HP:
                        # stagger second-half data loads (dnat slots free up
                        # as first halves retire)
                        emt = (i - 26) // 2
                        early_h[emt][1] = load_half(emt, 1)
                    if i >= AL:
                        phase_a_step(i - AL)
                for i in range(len(ablocks) - AL, len(ablocks)):
                    phase_a_step(i)
                for emt in range(PIPE):
                    emit_evict(emt, early_pmm[emt])
                if HALF:
                    emit_evict(PIPE, early_pmm[PIPE], nhs=(0,))

                # ---- Phase B: flat software pipeline over (m-tile, kc) ----
                # Matmuls lag transposes by LAG groups so the psum->sbuf
                # evictions complete well before their consumers, and the PE
                # stream alternates 4 transposes / 8 matmuls with no bursts at
                # m-tile boundaries.
                LAG = int(os.environ.get("KP_LAG", "3"))
                steady = ([(PIPE, kc, (1,)) for kc in range(KO // 4)]
                          if HALF else [])
                steady += [(mt, kc, (0, 1)) for mt in range(HP, MT)
                           for kc in range(KO // 4)]
                halves = {PIPE: [load_half(PIPE, 0), load_half(PIPE, 1)]}
                pmms = {}
                dTq = deque()

                def retire(idx):
                    pmt, pkc, pnhs = steady[idx]
                    emit_mms(dTq.popleft(), pkc, pmms[pmt], nhs=pnhs)
                    if pkc == KO // 4 - 1:
                        emit_evict(pmt, pmms.pop(pmt), nhs=pnhs)

                for idx, (mt, kc, nhs) in enumerate(steady):
                    if kc == 0:
                        pmms[mt] = alloc_pmm(nhs)
                    if kc == 2 and mt + 1 < MT:
                        halves[mt + 1] = [load_half(mt + 1, 0), None]
                    if kc == 6 and mt + 1 < MT:
                        halves[mt + 1][1] = load_half(mt + 1, 1)
                        halves.pop(mt - 1, None)
                    dTq.append(
                        emit_transposes(halves[mt][kc // 4], kc, use_act=idx % 2))
                    if idx >= LAG:
                        retire(idx - LAG)
                for idx in range(len(steady) - LAG, len(steady)):
                    retire(idx)

    nc.compile()
    return nc


_PROGRAM = None


def _build_trivial_program():
    nc = bacc.Bacc("TRN2", target_bir_lowering=False, debug=False,
                   num_devices=N_CORES)
    x_d = nc.dram_tensor("x", [P, 256], F32, kind="ExternalInput").ap()
    y_d = nc.dram_tensor("y", [P, 256], F32, kind="ExternalOutput").ap()
    with tile.TileContext(nc) as tc:
        with tc.tile_pool(name="sbuf", bufs=1) as pool:
            t = pool.tile([P, 256], F32, name="t")
            nc.sync.dma_start(t[:], x_d)
            nc.sync.dma_start(y_d, t[:])
    nc.compile()
    return nc


def _make_dispatch_fn(nc):
    """Zero-arg callable running one 8-core dispatch with device-resident
    zero inputs. Used only for timing."""
    import jax
    from jax.sharding import Mesh, PartitionSpec
    from jax.experimental.shard_map import shard_map
    from concourse import bass2jax, mybir as _mybir

    bass2jax.install_neuronx_cc_hook()

    in_names, out_names, out_avals, zero_shapes = [], [], [], []
    for alloc in nc.m.functions[0].allocations:
        if not isinstance(_mybir.MemoryLocationSet, type) or not isinstance(
                alloc, _mybir.MemoryLocationSet):
            continue
        name = alloc.memorylocations[0].name
        if alloc.kind == "ExternalInput":
            in_names.append((name, tuple(alloc.tensor_shape),
                             _mybir.dt.np(alloc.dtype)))
        elif alloc.kind == "ExternalOutput":
            out_names.append(name)
            shape = tuple(alloc.tensor_shape)
            dtype = _mybir.dt.np(alloc.dtype)
            out_avals.append(jax.core.ShapedArray(shape, dtype))
            zero_shapes.append((shape, dtype))
    n_params = len(in_names)
    all_names = [n for n, _, _ in in_names] + out_names

    def _body(*args):
        outs = bass2jax._bass_exec_p.bind(
            *args,
            out_avals=tuple(out_avals),
            in_names=tuple(all_names),
            out_names=tuple(out_names),
            lowering_input_output_aliases=(),
            sim_require_finite=True,
            sim_require_nnan=True,
            nc=nc,
        )
        return tuple(outs)

    devices = jax.devices()[:N_CORES]
    mesh = Mesh(np.asarray(devices), ("core",))
    n_all = n_params + len(out_names)
    fn = jax.jit(
        shard_map(_body, mesh=mesh,
                  in_specs=(PartitionSpec("core"),) * n_all,
                  out_specs=(PartitionSpec("core"),) * len(out_names),
                  check_rep=False),
        keep_unused=True,
    )
    sharding = jax.sharding.NamedSharding(mesh, PartitionSpec("core"))
    dev_in = [
        jax.device_put(
            np.zeros((N_CORES * shape[0], *shape[1:]), dtype), sharding)
        for _, shape, dtype in in_names
    ] + [
        jax.device_put(
            np.zeros((N_CORES * shape[0], *shape[1:]), dtype), sharding)
        for shape, dtype in zero_shapes
    ]
    return lambda: fn(*dev_in)


def measure_hw_time_ns(reps=30):
    """HW kernel time estimate: dispatch time minus trivial-NEFF dispatch
    time, sampled interleaved (the RPC floor drifts on the order of ms)."""
    import time as _time
    import jax

    global _PROGRAM
    if _PROGRAM is None:
        _PROGRAM = _build_program()
    fn_k = _make_dispatch_fn(_PROGRAM)
    fn_t = _make_dispatch_fn(_build_trivial_program())
    jax.block_until_ready(fn_k())
    jax.block_until_ready(fn_t())
    diffs = []
    for _ in range(reps):
        t0 = _time.perf_counter()
        jax.block_until_ready(fn_t())
        t1 = _time.perf_counter()
        jax.block_until_ready(fn_k())
        t2 = _time.perf_counter()
        jax.block_until_ready(fn_t())
        t3 = _time.perf_counter()
        # kernel minus mean of surrounding trivials cancels slow drift
        diffs.append((t2 - t1) - ((t1 - t0) + (t3 - t2)) / 2)
    diffs.sort()
    med = diffs[len(diffs) // 2]
    lo, hi = diffs[len(diffs) // 4], diffs[3 * len(diffs) // 4]
    print(f"[timing] kernel-minus-floor: median {med*1e3:.3f} ms "
          f"(IQR {lo*1e3:.3f}..{hi*1e3:.3f} ms, n={reps})")
    return int(med * 1e9)


def kernel(data, weight, w_mask, bias_p):
    global _PROGRAM, LAST_RESULT
    data = np.asarray(data, dtype=np.float32)
    weight = np.asarray(weight, dtype=np.float32)
    w_mask = np.asarray(w_mask, dtype=np.float32)
    bias_p = np.asarray(bias_p, dtype=np.float32)

    dataf = np.ascontiguousarray(data.reshape(M_TOT, K))

    if _PROGRAM is None:
        _PROGRAM = _build_program()
    nc = _PROGRAM

    in_maps = []
    for c in range(N_CORES):
        ns = c % N_SHARDS
        ms = c // N_SHARDS
        in_maps.append({
            "data": np.ascontiguousarray(dataf[ms * M_C:(ms + 1) * M_C]),
            "w": np.ascontiguousarray(weight[ns * N_C:(ns + 1) * N_C]),
            "mask": np.ascontiguousarray(w_mask[ns * N_C:(ns + 1) * N_C]),
            "bias": np.ascontiguousarray(
                np.tile(bias_p[ns * N_C:(ns + 1) * N_C][None, :], (P, 1))),
        })

    res = run_bass_kernel_spmd(nc, in_maps, core_ids=list(range(N_CORES)))
    LAST_RESULT = res

    out = np.empty((M_TOT, N_TOT), dtype=np.float32)
    for c in range(N_CORES):
        ns = c % N_SHARDS
        ms = c // N_SHARDS
        out[ms * M_C:(ms + 1) * M_C, ns * N_C:(ns + 1) * N_C] = \
            res.results[c]["out"]
    return out.reshape(4, 2048, N_TOT)

